# revision 1
# baseline (speedup 1.0000x reference)
"""Gated multi-head self-attention on 8 Trainium2 NeuronCores.

Sharding: 16 heads / 8 cores = 2 heads per core. Each core computes its two
heads end-to-end (QKV projection, attention, per-head norm, output
projection) and writes a partial [B*T, D] output; the host sums the 8
partials and adds the head-summed output bias.

Device algorithm per core (heads h0, h1), everything f32 with float32r
matmuls:
  QT/KT[128, 4096]  = W_{q,k}.T @ x.T + b   (both heads stacked on partitions)
  V'[s, 65]         = [x@W_v | 1]           (per head, via VT + PE transpose)
  S^T[s, q]         = KT.T @ QT             (heads packed in PE via tile_position)
  P^T               = exp(0.125 * S^T)      (no max subtraction: |scores| is tiny)
  [out^T; d]        = V'.T @ P^T            (row 64 = softmax denominators)
  out_sm            = out^T * bcast(1/d) + b_v    (softmax weights sum to 1 =>
                                                   V bias passes through additively)
  denom_h           = max(mean_t ||out_sm[:, t]||, 1e-5)
  proj             += (out_sm / denom_h).T @ (W_o * g/16)  summed over 2 heads
"""

import sys

sys.path.insert(0, "/opt/trn_rl_repo")

import contextlib

import numpy as np

import concourse.bacc as bacc
import concourse.mybir as mybir
import concourse.tile as tile
from concourse.bass_utils import run_bass_kernel_spmd
from concourse.masks import make_identity

f32 = mybir.dt.float32
f32r = mybir.dt.float32r
AF = mybir.ActivationFunctionType
ALU = mybir.AluOpType

B, T, D, H, HD = 2, 2048, 1024, 16, 64
NCORES = 8
HPC = H // NCORES  # heads per core = 2
NT = B * T         # 4096 tokens
SCALE = 1.0 / np.sqrt(HD)  # 0.125

_BUILD_CACHE = {}


def _build(with_mask: bool, repeat: int = 1):
    nc = bacc.Bacc(None, target_bir_lowering=False)

    xT = nc.declare_dram_parameter("xT", [D, NT], f32r, isOutput=False)
    wqkv = nc.declare_dram_parameter("wqkv", [3, 8, 128, 128], f32r, isOutput=False)
    bqk = nc.declare_dram_parameter("bqk", [2, 128], f32, isOutput=False)
    bv = nc.declare_dram_parameter("bv", [HPC, HD], f32, isOutput=False)
    wo = nc.declare_dram_parameter("wo", [HPC, HD, D], f32r, isOutput=False)
    outp = nc.declare_dram_parameter("outp", [NT, D], f32, isOutput=True)
    if with_mask:
        maskT = nc.declare_dram_parameter("maskT", [T, T], f32, isOutput=False)

    with tile.TileContext(nc) as tc, contextlib.ExitStack() as ctx:
        wp = ctx.enter_context(tc.tile_pool(name="wp", bufs=1))
        big = ctx.enter_context(tc.tile_pool(name="big", bufs=2))
        xp = ctx.enter_context(tc.tile_pool(name="xp", bufs=3 if with_mask else 4))
        vtp = ctx.enter_context(tc.tile_pool(name="vtp", bufs=2))
        pp = ctx.enter_context(tc.tile_pool(name="pp", bufs=4))
        rowp = ctx.enter_context(tc.tile_pool(name="rowp", bufs=2))
        auxp = ctx.enter_context(tc.tile_pool(name="auxp", bufs=2))
        nsqp = ctx.enter_context(tc.tile_pool(name="nsqp", bufs=2))
        osmp = ctx.enter_context(tc.tile_pool(name="osmp", bufs=2))
        scp = ctx.enter_context(tc.tile_pool(name="scp", bufs=6))
        op = ctx.enter_context(tc.tile_pool(name="op", bufs=2))
        if with_mask:
            mp = ctx.enter_context(tc.tile_pool(name="mp", bufs=2))
        psum = ctx.enter_context(tc.tile_pool(name="psum", bufs=4, space="PSUM"))

        # ---- constants / weights ----
        wqkv_sb = wp.tile([128, 3, 8, 128], f32r)
        nc.sync.dma_start(out=wqkv_sb[:], in_=wqkv.ap().rearrange("q d p m -> p q d m"))
        bqk_sb = wp.tile([128, 2], f32)
        nc.sync.dma_start(out=bqk_sb[:], in_=bqk.ap().rearrange("q p -> p q"))
        bv_sb = wp.tile([HD, HPC], f32)
        nc.sync.dma_start(out=bv_sb[:], in_=bv.ap().rearrange("h p -> p h"))
        wo_sb = wp.tile([HD, HPC, D], f32r)
        nc.sync.dma_start(out=wo_sb[:], in_=wo.ap().rearrange("h p d -> p h d"))
        ones_f = wp.tile([128, 1], f32)
        nc.vector.memset(ones_f[:], 1.0)
        ones64 = wp.tile([HD, 1], f32r)
        nc.vector.tensor_copy(ones64[:], ones_f[0:64, :])
        ident = wp.tile([128, 128], f32)
        make_identity(nc, ident[:])

        # V' [s-part, s-chunk, head, 66]: cols 0:64 = V, col 64 = ones, col 65 pad
        Vp = wp.tile([128, NT // 128, HPC, 66], f32r)
        nc.vector.tensor_copy(Vp[:, :, :, 64:65],
                              ones_f.broadcast_to([128, NT // 128, HPC, 1]))

        QT = big.tile([128, NT], f32r, tag="big")
        KT = big.tile([128, NT], f32r, tag="big")

        # ---- per-head state for phase C ----
        osm = [None, None]       # out_sm [64, NT] fp32
        nsq = [None, None]       # per-token squared norms [1, NT]
        for h in range(HPC):
            osm[h] = osmp.tile([HD, NT], f32, name=f"osm{h}", tag="osm")
            nsq[h] = nsqp.tile([1, NT], f32, name=f"nsq{h}", tag="nsq")

        def qkv_chunk(c8):
            """Project tokens [c8*512, (c8+1)*512) -> QT, KT cols; V' rows."""
            halves = []
            for hh in range(2):
                xs = xp.tile([128, 4, 512], f32r, tag="xslab", name=f"xs{hh}")
                # one DMA per 256KB d-chunk so transfers spread across queues
                for dd in range(4):
                    nc.sync.dma_start(
                        out=xs[:, dd, :],
                        in_=xT.ap()[:, c8 * 512:(c8 + 1) * 512]
                        .rearrange("(dc p) t -> p dc t", p=128)[:, hh * 4 + dd, :])
                halves.append(xs)
            def xsl(dc):
                return halves[dc // 4][:, dc % 4, :]
            cols = slice(c8 * 512, (c8 + 1) * 512)
            for p, dst in ((0, QT), (1, KT)):
                ps = psum.tile([128, 512], f32, tag="a", name="ps_qkv")
                for dc in range(8):
                    nc.tensor.matmul(ps[:], wqkv_sb[:, p, dc, :], xsl(dc),
                                     start=(dc == 0), stop=(dc == 7))
                # rounds to f32r on write; adds per-partition bias
                nc.vector.tensor_scalar_add(dst[:, cols], ps[:], bqk_sb[:, p:p + 1])
            # V projection -> VT chunk [128(hd2), 512]
            psv = psum.tile([128, 512], f32, tag="a", name="ps_v")
            for dc in range(8):
                nc.tensor.matmul(psv[:], wqkv_sb[:, 2, dc, :], xsl(dc),
                                 start=(dc == 0), stop=(dc == 7))
            vt = vtp.tile([128, 512], f32, tag="vt")
            nc.scalar.activation(vt[:], psv[:], AF.Copy)
            # transpose VT -> V' (per head, 4 s-tiles of 128)
            for s4 in range(4):
                j = c8 * 4 + s4
                for h in range(HPC):
                    pt = psum.tile([128, 64], f32, tag="a", name="ps_tr")
                    nc.tensor.transpose(
                        pt[:], vt[h * 64:(h + 1) * 64, s4 * 128:(s4 + 1) * 128],
                        ident[h * 64:(h + 1) * 64, h * 64:(h + 1) * 64])
                    nc.vector.tensor_copy(Vp[:, j, h, 0:64], pt[:])

        def attn_qc(b, qc):
            """One 512-query chunk of attention for batch b, both heads."""
            qcols = slice(b * T + qc * 512, b * T + (qc + 1) * 512)
            po = [psum.tile([65, 512], f32, tag="a", name=f"po{h}") for h in range(HPC)]
            NJ = T // 128
            prev_pe = None

            def av(j, pe):
                for h in range(HPC):
                    nc.tensor.matmul(po[h][:], Vp[:, b * NJ + j, h, 0:65],
                                     pe[:, h * 512:(h + 1) * 512],
                                     start=(j == 0), stop=(j == NJ - 1))

            for j in range(NJ):
                scols = slice(b * T + j * 128, b * T + (j + 1) * 128)
                s2 = psum.tile([128, 1024], f32, tag="s2", bufs=2, name="s2")
                for h in range(HPC):
                    nc.tensor.matmul(s2[:, h * 512:(h + 1) * 512],
                                     KT[h * 64:(h + 1) * 64, scols],
                                     QT[h * 64:(h + 1) * 64, qcols],
                                     start=True, stop=True,
                                     tile_position=(h * 64, 0))
                pe = pp.tile([128, 1024], f32r, tag="p")
                if with_mask:
                    mt = mp.tile([128, 512], f32, tag="m")
                    nc.sync.dma_start(
                        out=mt[:],
                        in_=maskT.ap()[j * 128:(j + 1) * 128,
                                       qc * 512:(qc + 1) * 512])
                    tmp = pp.tile([128, 1024], f32, tag="tmp", bufs=2)
                    for h in range(HPC):
                        nc.vector.scalar_tensor_tensor(
                            tmp[:, h * 512:(h + 1) * 512],
                            s2[:, h * 512:(h + 1) * 512], SCALE, mt[:],
                            op0=ALU.mult, op1=ALU.add)
                    nc.scalar.activation(pe[:], tmp[:], AF.Exp)
                else:
                    nc.scalar.activation(pe[:], s2[:], AF.Exp, scale=float(SCALE))
                if prev_pe is not None:
                    av(j - 1, prev_pe)
                prev_pe = pe
            av(NJ - 1, prev_pe)
            # phase-C chunk work, inline: normalize by softmax denom, add bv
            for h in range(HPC):
                o65 = auxp.tile([65, 512], f32, tag="o65", bufs=2)
                nc.vector.tensor_copy(o65[:], po[h][:])
                drow = rowp.tile([1, 512], f32, tag="row")
                nc.sync.dma_start(out=drow[:], in_=o65[64:65, :])
                rrow = rowp.tile([1, 512], f32, tag="row")
                nc.vector.reciprocal(rrow[:], drow[:])
                bc = auxp.tile([HD, 512], f32, tag="aux")
                nc.gpsimd.partition_broadcast(bc[:], rrow[:])
                t1 = auxp.tile([HD, 512], f32, tag="aux")
                nc.vector.tensor_tensor(t1[:], o65[0:64, :], bc[:], op=ALU.mult)
                oc = osm[h][:, qcols]
                nc.vector.tensor_scalar_add(oc, t1[:], bv_sb[:, h:h + 1])
                # running norm sums: sq -> column sums (PE) -> sqrt -> row sum
                sq = auxp.tile([HD, 512], f32r, tag="aux")
                nc.vector.tensor_tensor(sq[:], oc, oc, op=ALU.mult)
                pn = psum.tile([1, 512], f32, tag="a", name="ps_n")
                nc.tensor.matmul(pn[:], ones64[:], sq[:], start=True, stop=True)
                nc.vector.tensor_copy(nsq[h][:, qcols], pn[:])

        def _emit_all():
            for c8 in range(4):
                qkv_chunk(c8)
            for qc in range(4):
                attn_qc(0, qc)
            for c8 in range(4, 8):
                qkv_chunk(c8)
            for qc in range(4):
                attn_qc(1, qc)

            # ---- finalize per-head scale, apply, project ----
            onn = [None, None]
            for h in range(HPC):
                onn[h] = big.tile([HD, NT], f32r, tag="big", name=f"onn{h}")
                tot = scp.tile([1, 1], f32, tag="sc")
                nc.scalar.activation(onn[h][0:1, :], nsq[h][:], AF.Sqrt,
                                     accum_out=tot[:])
                den = scp.tile([1, 1], f32, tag="sc")
                nc.vector.tensor_scalar(den[:], tot[:], 1.0 / NT, 1e-5,
                                        op0=ALU.mult, op1=ALU.max)
                inv = scp.tile([1, 1], f32, tag="sc")
                nc.vector.reciprocal(inv[:], den[:])
                inv64 = scp.tile([HD, 1], f32, tag="sc64")
                nc.gpsimd.partition_broadcast(inv64[:], inv[:])
                nc.vector.tensor_scalar(onn[h][:], osm[h][:], inv64[:], None, op0=ALU.mult)

            for t in range(NT // 128):
                trows = slice(t * 128, (t + 1) * 128)
                for dchunk in range(2):
                    dcols = slice(dchunk * 512, (dchunk + 1) * 512)
                    ppj = psum.tile([128, 512], f32, tag="a", name="ps_p")
                    for h in range(HPC):
                        nc.tensor.matmul(ppj[:], onn[h][:, trows], wo_sb[:, h, dcols],
                                         start=(h == 0), stop=(h == HPC - 1))
                    osb = op.tile([128, 512], f32, tag="ob")
                    nc.vector.tensor_copy(osb[:], ppj[:])
                    nc.sync.dma_start(out=outp.ap()[trows, dcols], in_=osb[:])

        if repeat > 1:
            with tc.For_i(0, repeat, 1):
                _emit_all()
        else:
            _emit_all()

    nc.compile()
    return nc


def _get_nc(with_mask: bool):
    key = with_mask
    if key not in _BUILD_CACHE:
        _BUILD_CACHE[key] = _build(with_mask)
    return _BUILD_CACHE[key]


def kernel(hidden_states, attn_mask, W_q, b_q, W_k, b_k, W_v, b_v, W_o, b_o, gate):
    hidden_states = np.asarray(hidden_states, dtype=np.float32)
    attn_mask = np.asarray(attn_mask, dtype=np.float32)
    W_q, b_q = np.asarray(W_q, np.float32), np.asarray(b_q, np.float32)
    W_k, b_k = np.asarray(W_k, np.float32), np.asarray(b_k, np.float32)
    W_v, b_v = np.asarray(W_v, np.float32), np.asarray(b_v, np.float32)
    W_o, b_o = np.asarray(W_o, np.float32), np.asarray(b_o, np.float32)
    gate = np.asarray(gate, np.float32)

    with_mask = bool(np.any(attn_mask))
    nc = _get_nc(with_mask)

    x = hidden_states.reshape(NT, D)
    xT = np.ascontiguousarray(x.T)
    g = np.clip(gate, 0.0, 1.0)

    in_maps = []
    for c in range(NCORES):
        hs = slice(c * HPC, (c + 1) * HPC)
        wq = np.concatenate([W_q[c * HPC + i] for i in range(HPC)], axis=1)  # [D, 128]
        wk = np.concatenate([W_k[c * HPC + i] for i in range(HPC)], axis=1)
        wv = np.concatenate([W_v[c * HPC + i] for i in range(HPC)], axis=1)
        wqkv_c = np.ascontiguousarray(
            np.stack([wq, wk, wv], axis=0).reshape(3, 8, 128, 128))
        bqk_c = np.ascontiguousarray(np.stack(
            [np.concatenate([b_q[c * HPC + i] for i in range(HPC)]),
             np.concatenate([b_k[c * HPC + i] for i in range(HPC)])], axis=0))
        bv_c = np.ascontiguousarray(b_v[hs])                      # [2, 64]
        wo_c = np.ascontiguousarray(
            W_o[hs] * (g[hs, None, None] / H))                    # [2, 64, D]
        m = dict(xT=xT, wqkv=wqkv_c, bqk=bqk_c, bv=bv_c, wo=wo_c)
        if with_mask:
            m["maskT"] = np.ascontiguousarray(attn_mask.T)
        in_maps.append(m)

    res = run_bass_kernel_spmd(nc, in_maps, core_ids=list(range(NCORES)))
    if res.exec_time_ns is not None:
        print(f"HW exec time: {res.exec_time_ns} ns")

    out = np.zeros((NT, D), dtype=np.float32)
    for r in res.results:
        out += r["outp"]
    b_eff = (np.clip(gate, 0.0, 1.0)[:, None] * b_o).sum(axis=0) / H
    out += b_eff[None, :]
    return out.reshape(B, T, D)



# revision 12
# speedup vs baseline: 1.3475x; 1.3475x over previous
"""Gated multi-head self-attention on 8 Trainium2 NeuronCores.

Sharding: 16 heads / 8 cores = 2 heads per core. Each core computes its two
heads end-to-end and writes a partial [NT, D] output (fp16); the host sums
the 8 partials and adds the head-summed output bias.

Device algorithm per core (heads h0, h1), fp16 matmul inputs, f32 PSUM:
  QT/KT[128, 4096]   = W_{q,k}.T @ x.T + b     (heads stacked on partitions)
  V'[s, h, 66]       = [x@W_v | 1]             (per head, via VT + PE transpose)
  S^T[s, q]          = KT.T @ QT               (heads packed via tile_position)
  P                  = exp(0.125 * S^T)        (fp16; |scores| small, no max-sub)
  poT[q, 65]         = P.T @ V'  (transposed AV: full 128-wide PE, qs-major
                       accumulation into a slot-rotated 1-bank PSUM tile;
                       col 64 = softmax denominators, a per-partition scalar)
  ocT[q, e]          = poT[:,0:64] * (1/poT[:,64]) + bv   (one DVE op each)
  nsq[q]             = sum_e ocT^2             (DVE tensor_tensor_reduce)
  osm[e, q]          = ocT^T                   (PE transpose back)
  denom_h            = max(mean sqrt(nsq), 1e-5);  wo_sc = wo * gate/16 / denom
  out               += osm.T @ wo_sc  summed over 2 heads, stored fp16

Scheduling: AV+phase-C of query-chunk qc-1 and the QKV projection of batch 1
are interleaved into the j-loop of later attention chunks as filler thunks,
keeping PE dense while the Activation engine streams the exp() softmax.
"""

import sys

sys.path.insert(0, "/opt/trn_rl_repo")

import contextlib

import numpy as np

import concourse.bacc as bacc
import concourse.mybir as mybir
import concourse.tile as tile
from concourse.bass_utils import run_bass_kernel_spmd
from concourse.masks import make_identity

f32 = mybir.dt.float32
f32r = mybir.dt.float32r
f16 = mybir.dt.float16
AF = mybir.ActivationFunctionType
ALU = mybir.AluOpType

B, T, D, H, HD = 2, 2048, 1024, 16, 64
NCORES = 8
HPC = H // NCORES  # heads per core = 2
NT = B * T         # 4096 tokens
NJ = T // 128      # 16 key tiles per batch
SCALE = 1.0 / np.sqrt(HD)  # 0.125

_BUILD_CACHE = {}


def _build(with_mask: bool, repeat: int = 1):
    nc = bacc.Bacc(None, target_bir_lowering=False)

    xT = nc.declare_dram_parameter("xT", [D, NT], f16, isOutput=False)
    wqkv = nc.declare_dram_parameter("wqkv", [3, 8, 128, 128], f16, isOutput=False)
    bqk = nc.declare_dram_parameter("bqk", [2, 128], f32, isOutput=False)
    bvr = nc.declare_dram_parameter("bvr", [1, 128], f32, isOutput=False)
    wo = nc.declare_dram_parameter("wo", [HPC, HD, D], f16, isOutput=False)
    outp = nc.declare_dram_parameter("outp", [NT, D], f16, isOutput=True)
    if with_mask:
        maskT = nc.declare_dram_parameter("maskT", [T, T], f32, isOutput=False)

    with tile.TileContext(nc) as tc, contextlib.ExitStack() as ctx:
        wp = ctx.enter_context(tc.tile_pool(name="wp", bufs=1))
        big = ctx.enter_context(tc.tile_pool(name="big", bufs=2))
        osmp = ctx.enter_context(tc.tile_pool(name="osmp", bufs=2))
        nsqp = ctx.enter_context(tc.tile_pool(name="nsqp", bufs=2))
        xp = ctx.enter_context(tc.tile_pool(name="xp", bufs=4))
        vtp = ctx.enter_context(tc.tile_pool(name="vtp", bufs=2))
        pp = ctx.enter_context(tc.tile_pool(name="pp", bufs=33))
        auxp = ctx.enter_context(tc.tile_pool(name="auxp", bufs=4))
        rowp = ctx.enter_context(tc.tile_pool(name="rowp", bufs=4))
        scp = ctx.enter_context(tc.tile_pool(name="scp", bufs=4))
        op = ctx.enter_context(tc.tile_pool(name="op", bufs=3))
        if with_mask:
            mp = ctx.enter_context(tc.tile_pool(name="mp", bufs=2))
            tmpp = ctx.enter_context(tc.tile_pool(name="tmpp", bufs=2))
        s2p = ctx.enter_context(tc.tile_pool(name="s2p", bufs=2, space="PSUM"))
        pop = ctx.enter_context(tc.tile_pool(name="pop", bufs=1, space="PSUM"))
        miscp = ctx.enter_context(tc.tile_pool(name="miscp", bufs=3, space="PSUM"))

        # ---- constants / weights ----
        wqkv_sb = wp.tile([128, 3, 8, 128], f16)
        nc.sync.dma_start(out=wqkv_sb[:], in_=wqkv.ap().rearrange("q d p m -> p q d m"))
        bqk_sb = wp.tile([128, 2], f32)
        nc.sync.dma_start(out=bqk_sb[:], in_=bqk.ap().rearrange("q p -> p q"))
        bvrow = wp.tile([1, 128], f32)
        nc.sync.dma_start(out=bvrow[:], in_=bvr.ap())
        bvb = wp.tile([128, 128], f32)  # [:, h*64:(h+1)*64] = bv_h bcast over parts
        nc.gpsimd.partition_broadcast(bvb[:], bvrow[:])
        wo_sb = wp.tile([HD, HPC, D], f16)
        nc.sync.dma_start(out=wo_sb[:], in_=wo.ap().rearrange("h p d -> p h d"))
        wo_sc = wp.tile([HD, HPC, D], f16)
        ones_f = wp.tile([128, 1], f32)
        nc.vector.memset(ones_f[:], 1.0)
        ones_h = wp.tile([128, 1], f16)
        nc.vector.tensor_copy(ones_h[:], ones_f[:])
        identb = wp.tile([128, 128], f16)
        make_identity(nc, identb[:])

        # V' [s-part, s-tile, head, 66]: cols 0:64 = V, col 64 = ones, 65 pad
        Vp = wp.tile([128, NT // 128, HPC, 66], f16)
        nc.vector.tensor_copy(Vp[:, :, :, 64:65],
                              ones_f.broadcast_to([128, NT // 128, HPC, 1]))

        QT = big.tile([128, NT], f16, tag="big")
        KT = big.tile([128, NT], f16, tag="big")

        # transposed-AV accumulator: 4 rotating slots of [*, 66] in one bank
        poT = pop.tile([128, 4, 66], f32, name="poT")

        osm = [None, None]        # out_sm^T [64, NT] f16 per head
        nsq = [None, None]        # per-token squared norms [1, NT]
        for h in range(HPC):
            osm[h] = osmp.tile([HD, NT], f16, name=f"osm{h}", tag="osm")
            nsq[h] = nsqp.tile([1, NT], f16, name=f"nsq{h}", tag="nsq")

        state = {"slot": 0}
        pe_store = {}

        def qkv_thunks(c4):
            """Emit-thunks projecting tokens [c4*1024, (c4+1)*1024)."""
            thunks = []
            xsh = {}

            def dma_thunk():
                for hh in range(2):
                    xs = xp.tile([128, 4, 1024], f16, tag="xs", name=f"xs{hh}")
                    for dd in range(4):
                        nc.sync.dma_start(
                            out=xs[:, dd, :],
                            in_=xT.ap()[:, c4 * 1024:(c4 + 1) * 1024]
                            .rearrange("(dc p) t -> p dc t", p=128)[:, hh * 4 + dd, :])
                    xsh[hh] = xs
            thunks.append(dma_thunk)

            def xsl(dc, cols):
                return xsh[dc // 4][:, dc % 4, cols]

            for sub in range(2):
                scols = slice(sub * 512, (sub + 1) * 512)
                gcols = slice(c4 * 1024 + sub * 512, c4 * 1024 + (sub + 1) * 512)

                def qk_proj(p, scols=scols, gcols=gcols):
                    dst = QT if p == 0 else KT
                    ps = miscp.tile([128, 512], f32, tag="a", name="ps_qk")
                    for dc in range(8):
                        nc.tensor.matmul(ps[:], wqkv_sb[:, p, dc, :], xsl(dc, scols),
                                         start=(dc == 0), stop=(dc == 7))
                    nc.vector.tensor_scalar_add(dst[:, gcols], ps[:],
                                                bqk_sb[:, p:p + 1])
                thunks.append(lambda p=0, f=qk_proj: f(p))
                thunks.append(lambda p=1, f=qk_proj: f(p))

                def v_proj(c4=c4, sub=sub, scols=scols):
                    psv = miscp.tile([128, 512], f32, tag="a", name="ps_v")
                    for dc in range(8):
                        nc.tensor.matmul(psv[:], wqkv_sb[:, 2, dc, :], xsl(dc, scols),
                                         start=(dc == 0), stop=(dc == 7))
                    vt = vtp.tile([128, 512], f16, tag="vt", name="vt")
                    nc.vector.tensor_copy(vt[:], psv[:])
                    xsh[f"vt{sub}"] = vt
                thunks.append(v_proj)

                def v_tr(s4, c4=c4, sub=sub):
                    vt = xsh[f"vt{sub}"]
                    j = c4 * 8 + sub * 4 + s4
                    for h in range(HPC):
                        pt = miscp.tile([128, 1024], f16, tag="a", name="ps_tr")
                        nc.tensor.transpose(
                            pt[:, 0:64],
                            vt[h * 64:(h + 1) * 64, s4 * 128:(s4 + 1) * 128],
                            identb[h * 64:(h + 1) * 64, h * 64:(h + 1) * 64])
                        nc.vector.tensor_copy(Vp[:, j, h, 0:64], pt[:, 0:64])
                for s4 in range(4):
                    thunks.append(lambda s4=s4, f=v_tr: f(s4))
            return thunks  # 15 thunks

        def attn_j(b, qc, fillers=()):
            """Scores+exp j-loop for one 512-query chunk; interleave fillers."""
            qcols = slice(b * T + qc * 512, b * T + (qc + 1) * 512)
            pes = []
            nf = len(fillers)
            fi = 0
            for j in range(NJ):
                scols = slice(b * T + j * 128, b * T + (j + 1) * 128)
                s2 = s2p.tile([128, 1024], f32, tag="s2", name="s2")
                for h in range(HPC):
                    nc.tensor.matmul(s2[:, h * 512:(h + 1) * 512],
                                     KT[h * 64:(h + 1) * 64, scols],
                                     QT[h * 64:(h + 1) * 64, qcols],
                                     start=True, stop=True,
                                     tile_position=(h * 64, 0))
                pe = pp.tile([128, 1024], f16, tag="p", name="pe")
                if with_mask:
                    mt = mp.tile([128, 512], f32, tag="m", name="mt")
                    nc.sync.dma_start(
                        out=mt[:],
                        in_=maskT.ap()[j * 128:(j + 1) * 128,
                                       qc * 512:(qc + 1) * 512])
                    tmp = tmpp.tile([128, 1024], f32, tag="tmp", name="tmp")
                    for h in range(HPC):
                        nc.vector.scalar_tensor_tensor(
                            tmp[:, h * 512:(h + 1) * 512],
                            s2[:, h * 512:(h + 1) * 512], SCALE, mt[:],
                            op0=ALU.mult, op1=ALU.add)
                    nc.scalar.activation(pe[:], tmp[:], AF.Exp)
                else:
                    nc.scalar.activation(pe[:], s2[:], AF.Exp, scale=float(SCALE))
                pes.append(pe)
                # interleave filler thunks evenly across the j-loop
                want = (j + 1) * nf // NJ
                while fi < want:
                    fillers[fi]()
                    fi += 1
            pe_store[(b, qc)] = pes

        def attn_consume(b, qc):
            """AV + phase-C thunks for chunk (b,qc): one per (h, qs)."""
            pes = pe_store.pop((b, qc))
            thunks = []

            def consume(h, qs):
                slot = state["slot"]
                state["slot"] = (slot + 1) % 4
                for j in range(NJ):
                    nc.tensor.matmul(
                        poT[:, slot, 0:65],
                        pes[j][:, h * 512 + qs * 128: h * 512 + (qs + 1) * 128],
                        Vp[:, b * NJ + j, h, 0:65],
                        start=(j == 0), stop=(j == NJ - 1))
                rinv = rowp.tile([128, 1], f32, tag="r", name="rinv")
                nc.vector.reciprocal(rinv[:], poT[:, slot, 64:65])
                ocT = auxp.tile([128, 64], f16, tag="oc", name="ocT")
                nc.vector.scalar_tensor_tensor(
                    ocT[:], poT[:, slot, 0:64], rinv[:],
                    bvb[:, h * 64:(h + 1) * 64],
                    op0=ALU.mult, op1=ALU.add)
                pt2 = miscp.tile([128, 1024], f16, tag="a", name="ps_oc")
                nc.tensor.transpose(pt2[0:64, 0:128], ocT[:], identb[:, :])
                tok = b * T + qc * 512 + qs * 128
                osmc = osm[h][:, tok:tok + 128]
                nc.vector.tensor_copy(osmc, pt2[0:64, 0:128])
                # per-token squared norms: e is on partitions now, so square
                # on DVE and column-sum with a ones-matmul (as in v1)
                sq = auxp.tile([HD, 128], f16, tag="sq", name="sq")
                nc.vector.tensor_tensor(sq[:], osmc, osmc, op=ALU.mult)
                pn = miscp.tile([128, 512], f32, tag="a", name="ps_n")
                nc.tensor.matmul(pn[0:1, 0:128], ones_h[0:HD, :], sq[:],
                                 start=True, stop=True)
                nc.vector.tensor_copy(nsq[h][:, tok:tok + 128], pn[0:1, 0:128])

            for h in range(HPC):
                for qs in range(4):
                    thunks.append(lambda h=h, qs=qs: consume(h, qs))
            return thunks

        def tail():
            for h in range(HPC):
                s1 = scp.tile([1, NT], f16, tag="s1", bufs=1, name="s1")
                tot = scp.tile([1, 1], f32, tag="c1", name="tot")
                nc.scalar.activation(s1[:], nsq[h][:], AF.Sqrt, accum_out=tot[:])
                den = scp.tile([1, 1], f32, tag="c1", name="den")
                nc.vector.tensor_scalar(den[:], tot[:], 1.0 / NT, 1e-5,
                                        op0=ALU.mult, op1=ALU.max)
                inv = scp.tile([1, 1], f32, tag="c1", name="inv")
                nc.vector.reciprocal(inv[:], den[:])
                inv64 = scp.tile([HD, 1], f32, tag="c64", name="inv64")
                nc.gpsimd.partition_broadcast(inv64[:], inv[:])
                nc.vector.tensor_scalar(wo_sc[:, h, :], wo_sb[:, h, :], inv64[:],
                                        None, op0=ALU.mult)
            for t in range(NT // 128):
                trows = slice(t * 128, (t + 1) * 128)
                osb = op.tile([128, D], f16, tag="ob", name="osb")
                for dchunk in range(2):
                    dcols = slice(dchunk * 512, (dchunk + 1) * 512)
                    ppj = miscp.tile([128, 512], f32, tag="a", name="ps_p")
                    for h in range(HPC):
                        nc.tensor.matmul(ppj[:], osm[h][:, trows],
                                         wo_sc[:, h, dcols],
                                         start=(h == 0), stop=(h == HPC - 1))
                    if dchunk == 0:
                        nc.vector.tensor_copy(osb[:, dcols], ppj[:])
                    else:
                        nc.scalar.activation(osb[:, dcols], ppj[:], AF.Copy)
                nc.sync.dma_start(out=outp.ap()[trows, :], in_=osb[:])

        def _emit_all():
            state["slot"] = 0
            ch = [qkv_thunks(c) for c in range(4)]
            for t in ch[0]:
                t()
            for t in ch[1]:
                t()
            attn_j(0, 0, fillers=ch[2][:8])
            cons = attn_consume(0, 0)
            attn_j(0, 1, fillers=cons + ch[2][8:])
            cons = attn_consume(0, 1)
            attn_j(0, 2, fillers=cons + ch[3][:8])
            cons = attn_consume(0, 2)
            attn_j(0, 3, fillers=cons + ch[3][8:])
            cons = attn_consume(0, 3)
            attn_j(1, 0, fillers=cons)
            cons = attn_consume(1, 0)
            attn_j(1, 1, fillers=cons)
            cons = attn_consume(1, 1)
            attn_j(1, 2, fillers=cons)
            cons = attn_consume(1, 2)
            attn_j(1, 3, fillers=cons)
            for t in attn_consume(1, 3):
                t()
            tail()

        if repeat > 1:
            with tc.For_i(0, repeat, 1):
                _emit_all()
        else:
            _emit_all()

    nc.compile()
    return nc


def _get_nc(with_mask: bool):
    key = with_mask
    if key not in _BUILD_CACHE:
        _BUILD_CACHE[key] = _build(with_mask)
    return _BUILD_CACHE[key]


def _make_in_maps(hidden_states, attn_mask, W_q, b_q, W_k, b_k, W_v, b_v,
                  W_o, b_o, gate, with_mask):
    x = hidden_states.reshape(NT, D)
    xT = np.ascontiguousarray(x.T.astype(np.float16))
    g = np.clip(gate, 0.0, 1.0)

    in_maps = []
    for c in range(NCORES):
        hs = slice(c * HPC, (c + 1) * HPC)
        wq = np.concatenate([W_q[c * HPC + i] for i in range(HPC)], axis=1)
        wk = np.concatenate([W_k[c * HPC + i] for i in range(HPC)], axis=1)
        wv = np.concatenate([W_v[c * HPC + i] for i in range(HPC)], axis=1)
        wqkv_c = np.ascontiguousarray(
            np.stack([wq, wk, wv], axis=0).reshape(3, 8, 128, 128)
            .astype(np.float16))
        bqk_c = np.ascontiguousarray(np.stack(
            [np.concatenate([b_q[c * HPC + i] for i in range(HPC)]),
             np.concatenate([b_k[c * HPC + i] for i in range(HPC)])], axis=0))
        bvr_c = np.ascontiguousarray(
            np.concatenate([b_v[c * HPC + i] for i in range(HPC)])[None, :])
        wo_c = np.ascontiguousarray(
            (W_o[hs] * (g[hs, None, None] / H)).astype(np.float16))
        m = dict(xT=xT, wqkv=wqkv_c, bqk=bqk_c, bvr=bvr_c, wo=wo_c)
        if with_mask:
            m["maskT"] = np.ascontiguousarray(attn_mask.T)
        in_maps.append(m)
    return in_maps


def kernel(hidden_states, attn_mask, W_q, b_q, W_k, b_k, W_v, b_v, W_o, b_o, gate):
    hidden_states = np.asarray(hidden_states, dtype=np.float32)
    attn_mask = np.asarray(attn_mask, dtype=np.float32)
    W_q, b_q = np.asarray(W_q, np.float32), np.asarray(b_q, np.float32)
    W_k, b_k = np.asarray(W_k, np.float32), np.asarray(b_k, np.float32)
    W_v, b_v = np.asarray(W_v, np.float32), np.asarray(b_v, np.float32)
    W_o, b_o = np.asarray(W_o, np.float32), np.asarray(b_o, np.float32)
    gate = np.asarray(gate, np.float32)

    with_mask = bool(np.any(attn_mask))
    nc = _get_nc(with_mask)
    in_maps = _make_in_maps(hidden_states, attn_mask, W_q, b_q, W_k, b_k,
                            W_v, b_v, W_o, b_o, gate, with_mask)

    res = run_bass_kernel_spmd(nc, in_maps, core_ids=list(range(NCORES)))
    if res.exec_time_ns is not None:
        print(f"HW exec time: {res.exec_time_ns} ns")

    out = np.zeros((NT, D), dtype=np.float32)
    for r in res.results:
        out += r["outp"].astype(np.float32)
    b_eff = (np.clip(gate, 0.0, 1.0)[:, None] * b_o).sum(axis=0) / H
    out += b_eff[None, :]
    return out.reshape(B, T, D)


# revision 15
# speedup vs baseline: 1.4266x; 1.0587x over previous
"""Gated multi-head self-attention on 8 Trainium2 NeuronCores.

Sharding: 16 heads / 8 cores = 2 heads per core. Each core computes its two
heads end-to-end and writes a partial [NT, D] output (fp16); the host sums
the 8 partials and adds the head-summed output bias.

Device algorithm per core (heads h0, h1), fp16 matmul front-end, f32 PSUM:
  QT/KT[128, 4096]   = W_{q,k}.T @ x.T + b     (f16; heads stacked on partitions)
  V'[s, h, 66]       = [x@W_v | 1]             (f32r, via VT + PE transpose)
  S^T[s, q]          = KT.T @ QT               (f16 in, heads packed in PE quads)
  P[s, q]            = exp(0.125 * S^T)        (f32r out: Act f16 writes run at
                                                half rate, f32r at full rate)
  po[65, q]          = V'.T @ P                (row 64 = softmax denominators)
  osm[e, q]          = po[0:64]·bcast(1/po[64]) + bv    (f16)
  nsq[q]             = ones.T @ osm^2          (per-token squared norms)
  denom_h            = max(mean sqrt(nsq), 1e-5);  wo_sc = wo * gate/16 / denom
  out               += osm.T @ wo_sc  summed over 2 heads, stored fp16

Scheduling: batch-1 QKV projection is interleaved into batch-0 attention
j-loops as filler thunks so PE stays dense while Act streams the exp()
softmax; phase C overlaps the next chunk's j-loop via the engine queues.
DMA queues: x loads, drow row-moves and output stores all on SP, which is
otherwise idle in those phases.
"""

import sys

sys.path.insert(0, "/opt/trn_rl_repo")

import contextlib

import numpy as np

import concourse.bacc as bacc
import concourse.mybir as mybir
import concourse.tile as tile
from concourse.bass_utils import run_bass_kernel_spmd
from concourse.masks import make_identity

f32 = mybir.dt.float32
f32r = mybir.dt.float32r
f16 = mybir.dt.float16
AF = mybir.ActivationFunctionType
ALU = mybir.AluOpType

B, T, D, H, HD = 2, 2048, 1024, 16, 64
NCORES = 8
HPC = H // NCORES  # heads per core = 2
NT = B * T         # 4096 tokens
NJ = T // 128      # 16 key tiles per batch
SCALE = 1.0 / np.sqrt(HD)  # 0.125

_BUILD_CACHE = {}


def _build(with_mask: bool, repeat: int = 1):
    nc = bacc.Bacc(None, target_bir_lowering=False)

    xT = nc.declare_dram_parameter("xT", [D, NT], f16, isOutput=False)
    wqkv = nc.declare_dram_parameter("wqkv", [3, 8, 128, 128], f16, isOutput=False)
    bqk = nc.declare_dram_parameter("bqk", [2, 128], f32, isOutput=False)
    bv = nc.declare_dram_parameter("bv", [HPC, HD], f32, isOutput=False)
    wo = nc.declare_dram_parameter("wo", [HPC, HD, D], f16, isOutput=False)
    outp = nc.declare_dram_parameter("outp", [NT, D], f16, isOutput=True)
    if with_mask:
        maskT = nc.declare_dram_parameter("maskT", [T, T], f32, isOutput=False)

    with tile.TileContext(nc) as tc, contextlib.ExitStack() as ctx:
        wp = ctx.enter_context(tc.tile_pool(name="wp", bufs=1))
        big = ctx.enter_context(tc.tile_pool(name="big", bufs=2))
        osmp = ctx.enter_context(tc.tile_pool(name="osmp", bufs=2))
        nsqp = ctx.enter_context(tc.tile_pool(name="nsqp", bufs=2))
        xp = ctx.enter_context(tc.tile_pool(name="xp", bufs=4))
        vtp = ctx.enter_context(tc.tile_pool(name="vtp", bufs=2))
        pp = ctx.enter_context(tc.tile_pool(name="pp", bufs=4))
        o65p = ctx.enter_context(tc.tile_pool(name="o65p", bufs=4))
        auxp = ctx.enter_context(tc.tile_pool(name="auxp", bufs=4))
        rowp = ctx.enter_context(tc.tile_pool(name="rowp", bufs=4))
        scp = ctx.enter_context(tc.tile_pool(name="scp", bufs=4))
        op = ctx.enter_context(tc.tile_pool(name="op", bufs=3))
        if with_mask:
            mp = ctx.enter_context(tc.tile_pool(name="mp", bufs=2))
            tmpp = ctx.enter_context(tc.tile_pool(name="tmpp", bufs=2))
        s2p = ctx.enter_context(tc.tile_pool(name="s2p", bufs=2, space="PSUM"))
        pot = ctx.enter_context(tc.tile_pool(name="pot", bufs=2, space="PSUM"))
        miscp = ctx.enter_context(tc.tile_pool(name="miscp", bufs=2, space="PSUM"))

        # ---- constants / weights ----
        wqkv_sb = wp.tile([128, 3, 8, 128], f16)
        nc.sync.dma_start(out=wqkv_sb[:], in_=wqkv.ap().rearrange("q d p m -> p q d m"))
        bqk_sb = wp.tile([128, 2], f32)
        nc.sync.dma_start(out=bqk_sb[:], in_=bqk.ap().rearrange("q p -> p q"))
        bv_sb = wp.tile([HD, HPC], f32)
        nc.sync.dma_start(out=bv_sb[:], in_=bv.ap().rearrange("h p -> p h"))
        wo_sb = wp.tile([HD, HPC, D], f16)
        nc.sync.dma_start(out=wo_sb[:], in_=wo.ap().rearrange("h p d -> p h d"))
        wo_sc = wp.tile([HD, HPC, D], f16)
        ones_f = wp.tile([128, 1], f32)
        nc.vector.memset(ones_f[:], 1.0)
        ones_r = wp.tile([128, 1], f32r)
        nc.vector.tensor_copy(ones_r[:], ones_f[:])
        identb = wp.tile([128, 128], f16)
        make_identity(nc, identb[:])

        # V' [s-part, s-tile, head, 66]: cols 0:64 = V, col 64 = ones, 65 pad
        Vp = wp.tile([128, NT // 128, HPC, 66], f32r)
        nc.vector.tensor_copy(Vp[:, :, :, 64:65],
                              ones_f.broadcast_to([128, NT // 128, HPC, 1]))

        QT = big.tile([128, NT], f16, tag="big")
        KT = big.tile([128, NT], f16, tag="big")

        osm = [None, None]        # out_sm^T [64, NT] f16 per head
        nsq = [None, None]        # per-token squared norms [1, NT] f16
        for h in range(HPC):
            osm[h] = osmp.tile([HD, NT], f16, name=f"osm{h}", tag="osm")
            nsq[h] = nsqp.tile([1, NT], f16, name=f"nsq{h}", tag="nsq")

        def qkv_thunks(c4):
            """Emit-thunks projecting tokens [c4*1024, (c4+1)*1024)."""
            thunks = []
            xsh = {}

            def dma_thunk():
                for hh in range(2):
                    xs = xp.tile([128, 4, 1024], f16, tag="xs", name=f"xs{hh}")
                    for dd in range(4):
                        nc.sync.dma_start(
                            out=xs[:, dd, :],
                            in_=xT.ap()[:, c4 * 1024:(c4 + 1) * 1024]
                            .rearrange("(dc p) t -> p dc t", p=128)[:, hh * 4 + dd, :])
                    xsh[hh] = xs
            thunks.append(dma_thunk)

            def xsl(dc, cols):
                return xsh[dc // 4][:, dc % 4, cols]

            for sub in range(2):
                scols = slice(sub * 512, (sub + 1) * 512)
                gcols = slice(c4 * 1024 + sub * 512, c4 * 1024 + (sub + 1) * 512)

                def qk_proj(p, scols=scols, gcols=gcols):
                    dst = QT if p == 0 else KT
                    ps = miscp.tile([128, 512], f32, tag="a", name="ps_qk")
                    for dc in range(8):
                        nc.tensor.matmul(ps[:], wqkv_sb[:, p, dc, :], xsl(dc, scols),
                                         start=(dc == 0), stop=(dc == 7))
                    nc.vector.tensor_scalar_add(dst[:, gcols], ps[:],
                                                bqk_sb[:, p:p + 1])
                thunks.append(lambda p=0, f=qk_proj: f(p))
                thunks.append(lambda p=1, f=qk_proj: f(p))

                def v_proj(c4=c4, sub=sub, scols=scols):
                    psv = miscp.tile([128, 512], f32, tag="a", name="ps_v")
                    for dc in range(8):
                        nc.tensor.matmul(psv[:], wqkv_sb[:, 2, dc, :], xsl(dc, scols),
                                         start=(dc == 0), stop=(dc == 7))
                    vt = vtp.tile([128, 512], f16, tag="vt", name="vt")
                    nc.vector.tensor_copy(vt[:], psv[:])
                    xsh[f"vt{sub}"] = vt
                thunks.append(v_proj)

                def v_tr(s4, c4=c4, sub=sub):
                    vt = xsh[f"vt{sub}"]
                    j = c4 * 8 + sub * 4 + s4
                    for h in range(HPC):
                        pt = miscp.tile([128, 1024], f16, tag="a", name="ps_tr")
                        nc.tensor.transpose(
                            pt[:, 0:64],
                            vt[h * 64:(h + 1) * 64, s4 * 128:(s4 + 1) * 128],
                            identb[h * 64:(h + 1) * 64, h * 64:(h + 1) * 64])
                        nc.vector.tensor_copy(Vp[:, j, h, 0:64], pt[:, 0:64])
                for s4 in range(4):
                    thunks.append(lambda s4=s4, f=v_tr: f(s4))
            return thunks  # 15 thunks

        def attn_qc(b, qc, fillers=()):
            """One 512-query attention chunk, j-major AV, inline phase C."""
            qcols = slice(b * T + qc * 512, b * T + (qc + 1) * 512)
            po = [pot.tile([65, 512], f32, tag="po", name=f"po{h}")
                  for h in range(HPC)]
            prev_pe = None
            prev_j = -1
            nf = len(fillers)
            fi = 0

            def av(j, pe):
                for h in range(HPC):
                    nc.tensor.matmul(po[h][:], Vp[:, b * NJ + j, h, 0:65],
                                     pe[:, h * 512:(h + 1) * 512],
                                     start=(j == 0), stop=(j == NJ - 1))

            for j in range(NJ):
                scols = slice(b * T + j * 128, b * T + (j + 1) * 128)
                s2 = s2p.tile([128, 1024], f32, tag="s2", name="s2")
                for h in range(HPC):
                    nc.tensor.matmul(s2[:, h * 512:(h + 1) * 512],
                                     KT[h * 64:(h + 1) * 64, scols],
                                     QT[h * 64:(h + 1) * 64, qcols],
                                     start=True, stop=True,
                                     tile_position=(h * 64, 0))
                pe = pp.tile([128, 1024], f32r, tag="p", name="pe")
                if with_mask:
                    mt = mp.tile([128, 512], f32, tag="m", name="mt")
                    nc.sync.dma_start(
                        out=mt[:],
                        in_=maskT.ap()[j * 128:(j + 1) * 128,
                                       qc * 512:(qc + 1) * 512])
                    tmp = tmpp.tile([128, 1024], f32, tag="tmp", name="tmp")
                    for h in range(HPC):
                        nc.vector.scalar_tensor_tensor(
                            tmp[:, h * 512:(h + 1) * 512],
                            s2[:, h * 512:(h + 1) * 512], SCALE, mt[:],
                            op0=ALU.mult, op1=ALU.add)
                    nc.scalar.activation(pe[:], tmp[:], AF.Exp)
                else:
                    nc.scalar.activation(pe[:], s2[:], AF.Exp, scale=float(SCALE))
                if prev_pe is not None:
                    av(prev_j, prev_pe)
                prev_pe, prev_j = pe, j
                want = (j + 1) * nf // NJ
                while fi < want:
                    fillers[fi]()
                    fi += 1
            av(prev_j, prev_pe)

            # phase C inline: po's only reader is the o65 copy, so po frees
            # early; the rest overlaps the next chunk's j-loop via the queues
            for h in range(HPC):
                o65 = o65p.tile([65, 512], f32, tag="o65", name="o65")
                nc.vector.tensor_copy(o65[:], po[h][:])
                drow = rowp.tile([1, 512], f32, tag="row", name="drow")
                nc.sync.dma_start(out=drow[:], in_=o65[64:65, :])
                rrow = rowp.tile([1, 512], f32, tag="row", name="rrow")
                nc.vector.reciprocal(rrow[:], drow[:])
                bc = auxp.tile([HD, 512], f32, tag="bc", name="bc")
                nc.gpsimd.partition_broadcast(bc[:], rrow[:])
                t1 = auxp.tile([HD, 512], f32r, tag="t1", name="t1")
                nc.vector.tensor_tensor(t1[:], o65[0:64, :], bc[:], op=ALU.mult)
                oc = osm[h][:, qcols]
                nc.vector.tensor_scalar_add(oc, t1[:], bv_sb[:, h:h + 1])
                sq = auxp.tile([HD, 512], f32r, tag="sq", name="sq")
                nc.vector.tensor_tensor(sq[:], oc, oc, op=ALU.mult)
                pn = miscp.tile([128, 512], f32, tag="a", name="ps_n")
                nc.tensor.matmul(pn[0:1, :], ones_r[0:HD, :], sq[:],
                                 start=True, stop=True)
                nc.vector.tensor_copy(nsq[h][:, qcols], pn[0:1, :])

        def tail():
            for h in range(HPC):
                s1 = scp.tile([1, NT], f16, tag="s1", bufs=1, name="s1")
                tot = scp.tile([1, 1], f32, tag="c1", name="tot")
                nc.scalar.activation(s1[:], nsq[h][:], AF.Sqrt, accum_out=tot[:])
                den = scp.tile([1, 1], f32, tag="c1", name="den")
                nc.vector.tensor_scalar(den[:], tot[:], 1.0 / NT, 1e-5,
                                        op0=ALU.mult, op1=ALU.max)
                inv = scp.tile([1, 1], f32, tag="c1", name="inv")
                nc.vector.reciprocal(inv[:], den[:])
                inv64 = scp.tile([HD, 1], f32, tag="c64", name="inv64")
                nc.gpsimd.partition_broadcast(inv64[:], inv[:])
                nc.vector.tensor_scalar(wo_sc[:, h, :], wo_sb[:, h, :], inv64[:],
                                        None, op0=ALU.mult)
            for t in range(NT // 128):
                trows = slice(t * 128, (t + 1) * 128)
                osb = op.tile([128, D], f16, tag="ob", name="osb")
                ppj = s2p.tile([128, 1024], f32, tag="s2", name="ps_p")
                for dchunk in range(2):
                    dcols = slice(dchunk * 512, (dchunk + 1) * 512)
                    for h in range(HPC):
                        nc.tensor.matmul(ppj[:, dcols], osm[h][:, trows],
                                         wo_sc[:, h, dcols],
                                         start=(h == 0), stop=(h == HPC - 1))
                if t % 2 == 0:
                    nc.vector.tensor_copy(osb[:], ppj[:])
                else:
                    nc.scalar.activation(osb[:], ppj[:], AF.Copy)
                nc.sync.dma_start(out=outp.ap()[trows, :], in_=osb[:])

        def _emit_all():
            ch = [qkv_thunks(c) for c in range(4)]
            for t in ch[0]:
                t()
            for t in ch[1]:
                t()
            attn_qc(0, 0, fillers=ch[2][:8])
            attn_qc(0, 1, fillers=ch[2][8:])
            attn_qc(0, 2, fillers=ch[3][:8])
            attn_qc(0, 3, fillers=ch[3][8:])
            for qc in range(4):
                attn_qc(1, qc)
            tail()

        if repeat > 1:
            with tc.For_i(0, repeat, 1):
                _emit_all()
        else:
            _emit_all()

    nc.compile()
    return nc


def _get_nc(with_mask: bool):
    key = with_mask
    if key not in _BUILD_CACHE:
        _BUILD_CACHE[key] = _build(with_mask)
    return _BUILD_CACHE[key]


def _make_in_maps(hidden_states, attn_mask, W_q, b_q, W_k, b_k, W_v, b_v,
                  W_o, b_o, gate, with_mask):
    x = hidden_states.reshape(NT, D)
    xT = np.ascontiguousarray(x.T.astype(np.float16))
    g = np.clip(gate, 0.0, 1.0)

    in_maps = []
    for c in range(NCORES):
        hs = slice(c * HPC, (c + 1) * HPC)
        wq = np.concatenate([W_q[c * HPC + i] for i in range(HPC)], axis=1)
        wk = np.concatenate([W_k[c * HPC + i] for i in range(HPC)], axis=1)
        wv = np.concatenate([W_v[c * HPC + i] for i in range(HPC)], axis=1)
        wqkv_c = np.ascontiguousarray(
            np.stack([wq, wk, wv], axis=0).reshape(3, 8, 128, 128)
            .astype(np.float16))
        bqk_c = np.ascontiguousarray(np.stack(
            [np.concatenate([b_q[c * HPC + i] for i in range(HPC)]),
             np.concatenate([b_k[c * HPC + i] for i in range(HPC)])], axis=0))
        bv_c = np.ascontiguousarray(b_v[hs])
        wo_c = np.ascontiguousarray(
            (W_o[hs] * (g[hs, None, None] / H)).astype(np.float16))
        m = dict(xT=xT, wqkv=wqkv_c, bqk=bqk_c, bv=bv_c, wo=wo_c)
        if with_mask:
            m["maskT"] = np.ascontiguousarray(attn_mask.T)
        in_maps.append(m)
    return in_maps


def kernel(hidden_states, attn_mask, W_q, b_q, W_k, b_k, W_v, b_v, W_o, b_o, gate):
    hidden_states = np.asarray(hidden_states, dtype=np.float32)
    attn_mask = np.asarray(attn_mask, dtype=np.float32)
    W_q, b_q = np.asarray(W_q, np.float32), np.asarray(b_q, np.float32)
    W_k, b_k = np.asarray(W_k, np.float32), np.asarray(b_k, np.float32)
    W_v, b_v = np.asarray(W_v, np.float32), np.asarray(b_v, np.float32)
    W_o, b_o = np.asarray(W_o, np.float32), np.asarray(b_o, np.float32)
    gate = np.asarray(gate, np.float32)

    with_mask = bool(np.any(attn_mask))
    nc = _get_nc(with_mask)
    in_maps = _make_in_maps(hidden_states, attn_mask, W_q, b_q, W_k, b_k,
                            W_v, b_v, W_o, b_o, gate, with_mask)

    res = run_bass_kernel_spmd(nc, in_maps, core_ids=list(range(NCORES)))
    if res.exec_time_ns is not None:
        print(f"HW exec time: {res.exec_time_ns} ns")

    out = np.zeros((NT, D), dtype=np.float32)
    for r in res.results:
        out += r["outp"].astype(np.float32)
    b_eff = (np.clip(gate, 0.0, 1.0)[:, None] * b_o).sum(axis=0) / H
    out += b_eff[None, :]
    return out.reshape(B, T, D)


# revision 20
# speedup vs baseline: 2.0719x; 1.4523x over previous
"""Gated multi-head self-attention on 8 Trainium2 NeuronCores.

Sharding: 16 heads / 8 cores = 2 heads per core. Each core computes its two
heads end-to-end and writes a partial [NT, D] output (fp16); the host sums
the 8 partials and adds the head-summed output bias.

Device algorithm per core (heads h0, h1), fp16 matmul front-end, f32 PSUM:
  QT/KT[128, 4096]   = W_{q,k}.T @ x.T + b     (f16; heads stacked on partitions)
  V'[s, h, 66]       = [x@W_v | 1]             (f32r, via VT + PE transpose)
  S^T[s, q]          = KT.T @ QT               (f16 in, heads packed in PE quads)
  P[s, q]            = exp(0.125 * S^T)        (f32r out: Act f16 writes run at
                                                half rate, f32r at full rate)
  po[65, q]          = V'.T @ P                (row 64 = softmax denominators)
  osm[e, q]          = po[0:64]·bcast(1/po[64]) + bv    (f16)
  nsq[q]             = ones.T @ osm^2          (per-token squared norms)
  denom_h            = max(mean sqrt(nsq), 1e-5);  wo_sc = wo * gate/16 / denom
  out               += osm.T @ wo_sc  summed over 2 heads, stored fp16

Scheduling: batch-1 QKV projection is interleaved into batch-0 attention
j-loops as filler thunks so PE stays dense while Act streams the exp()
softmax; phase C overlaps the next chunk's j-loop via the engine queues.
DMA queues: x loads, drow row-moves and output stores all on SP, which is
otherwise idle in those phases.
"""

import sys

sys.path.insert(0, "/opt/trn_rl_repo")

import contextlib

import numpy as np

import concourse.bacc as bacc
import concourse.mybir as mybir
import concourse.tile as tile
from concourse.bass_utils import run_bass_kernel_spmd
from concourse.masks import make_identity

f32 = mybir.dt.float32
f32r = mybir.dt.float32r
f16 = mybir.dt.float16
AF = mybir.ActivationFunctionType
ALU = mybir.AluOpType

B, T, D, H, HD = 2, 2048, 1024, 16, 64
NCORES = 8
HPC = H // NCORES  # heads per core = 2
NT = B * T         # 4096 tokens
NJ = T // 128      # 16 key tiles per batch
SCALE = 1.0 / np.sqrt(HD)  # 0.125

_BUILD_CACHE = {}


def _build(with_mask: bool, repeat: int = 1):
    nc = bacc.Bacc(None, target_bir_lowering=False)

    xT = nc.declare_dram_parameter("xT", [D, NT], f16, isOutput=False)
    wqkv = nc.declare_dram_parameter("wqkv", [3, 8, 128, 128], f16, isOutput=False)
    bqk = nc.declare_dram_parameter("bqk", [2, 128], f32, isOutput=False)
    bv = nc.declare_dram_parameter("bv", [HPC, HD], f32, isOutput=False)
    wo = nc.declare_dram_parameter("wo", [HPC, HD, D], f16, isOutput=False)
    selc = nc.declare_dram_parameter("selc", [128, 128], f16, isOutput=False)
    sel64d = nc.declare_dram_parameter("sel64d", [128, 2], f32r, isOutput=False)
    outp = nc.declare_dram_parameter("outp", [NT, D], f16, isOutput=True)
    if with_mask:
        maskT = nc.declare_dram_parameter("maskT", [T, T], f32, isOutput=False)

    with tile.TileContext(nc) as tc, contextlib.ExitStack() as ctx:
        wp = ctx.enter_context(tc.tile_pool(name="wp", bufs=1))
        big = ctx.enter_context(tc.tile_pool(name="big", bufs=2))
        osmp = ctx.enter_context(tc.tile_pool(name="osmp", bufs=2))
        nsqp = ctx.enter_context(tc.tile_pool(name="nsqp", bufs=2))
        xp = ctx.enter_context(tc.tile_pool(name="xp", bufs=4))
        vtp = ctx.enter_context(tc.tile_pool(name="vtp", bufs=2))
        pp = ctx.enter_context(tc.tile_pool(name="pp", bufs=4))
        o65p = ctx.enter_context(tc.tile_pool(name="o65p", bufs=4))
        auxp = ctx.enter_context(tc.tile_pool(name="auxp", bufs=4))
        rowp = ctx.enter_context(tc.tile_pool(name="rowp", bufs=4))
        scp = ctx.enter_context(tc.tile_pool(name="scp", bufs=4))
        op = ctx.enter_context(tc.tile_pool(name="op", bufs=3))
        if with_mask:
            mp = ctx.enter_context(tc.tile_pool(name="mp", bufs=2))
            tmpp = ctx.enter_context(tc.tile_pool(name="tmpp", bufs=2))
        s2p = ctx.enter_context(tc.tile_pool(name="s2p", bufs=2, space="PSUM"))
        pot = ctx.enter_context(tc.tile_pool(name="pot", bufs=2, space="PSUM"))
        miscp = ctx.enter_context(tc.tile_pool(name="miscp", bufs=2, space="PSUM"))

        # ---- constants / weights ----
        wqkv_sb = wp.tile([128, 3, 8, 128], f16)
        nc.sync.dma_start(out=wqkv_sb[:], in_=wqkv.ap().rearrange("q d p m -> p q d m"))
        bqk_sb = wp.tile([128, 2], f32)
        nc.sync.dma_start(out=bqk_sb[:], in_=bqk.ap().rearrange("q p -> p q"))
        bv_sb = wp.tile([HD, HPC], f32)
        nc.sync.dma_start(out=bv_sb[:], in_=bv.ap().rearrange("h p -> p h"))
        wo_sb = wp.tile([128, D], f16)
        nc.sync.dma_start(out=wo_sb[:], in_=wo.ap().rearrange("h p d -> (h p) d"))
        wo_sc = wp.tile([128, D], f16)
        ones_f = wp.tile([128, 1], f32)
        nc.vector.memset(ones_f[:], 1.0)
        sel64 = wp.tile([128, 2], f32r)
        nc.sync.dma_start(out=sel64[:], in_=sel64d.ap())
        selbc = wp.tile([128, 128], f16)
        nc.sync.dma_start(out=selbc[:], in_=selc.ap())
        identb = wp.tile([128, 128], f16)
        make_identity(nc, identb[:])

        # V' [s-part, s-tile, head, 66]: cols 0:64 = V, col 64 = ones, 65 pad
        Vp = wp.tile([128, NT // 128, HPC, 66], f32r)
        nc.vector.tensor_copy(Vp[:, :, :, 64:65],
                              ones_f.broadcast_to([128, NT // 128, HPC, 1]))

        QT = big.tile([128, NT], f16, tag="big")
        KT = big.tile([128, NT], f16, tag="big")

        osm2 = osmp.tile([128, NT], f16, name="osm2", tag="osm")
        nsq2 = nsqp.tile([2, NT], f16, name="nsq2", tag="nsq")

        def qkv_thunks(c4):
            """Emit-thunks projecting tokens [c4*1024, (c4+1)*1024)."""
            thunks = []
            xsh = {}

            def dma_thunk():
                for hh in range(2):
                    xs = xp.tile([128, 4, 1024], f16, tag="xs", name=f"xs{hh}")
                    for dd in range(4):
                        nc.sync.dma_start(
                            out=xs[:, dd, :],
                            in_=xT.ap()[:, c4 * 1024:(c4 + 1) * 1024]
                            .rearrange("(dc p) t -> p dc t", p=128)[:, hh * 4 + dd, :])
                    xsh[hh] = xs
            thunks.append(dma_thunk)

            def xsl(dc, cols):
                return xsh[dc // 4][:, dc % 4, cols]

            for sub in range(2):
                scols = slice(sub * 512, (sub + 1) * 512)
                gcols = slice(c4 * 1024 + sub * 512, c4 * 1024 + (sub + 1) * 512)

                def qk_proj(p, scols=scols, gcols=gcols):
                    dst = QT if p == 0 else KT
                    ps = miscp.tile([128, 512], f32, tag="a", name="ps_qk")
                    for dc in range(8):
                        nc.tensor.matmul(ps[:], wqkv_sb[:, p, dc, :], xsl(dc, scols),
                                         start=(dc == 0), stop=(dc == 7))
                    nc.vector.tensor_scalar_add(dst[:, gcols], ps[:],
                                                bqk_sb[:, p:p + 1])
                thunks.append(lambda p=0, f=qk_proj: f(p))
                thunks.append(lambda p=1, f=qk_proj: f(p))

                def v_proj(c4=c4, sub=sub, scols=scols):
                    psv = miscp.tile([128, 512], f32, tag="a", name="ps_v")
                    for dc in range(8):
                        nc.tensor.matmul(psv[:], wqkv_sb[:, 2, dc, :], xsl(dc, scols),
                                         start=(dc == 0), stop=(dc == 7))
                    vt = vtp.tile([128, 512], f16, tag="vt", name="vt")
                    nc.vector.tensor_copy(vt[:], psv[:])
                    xsh[f"vt{sub}"] = vt
                thunks.append(v_proj)

                def v_tr(s4, c4=c4, sub=sub):
                    vt = xsh[f"vt{sub}"]
                    j = c4 * 8 + sub * 4 + s4
                    for h in range(HPC):
                        pt = miscp.tile([128, 1024], f16, tag="a", name="ps_tr")
                        nc.tensor.transpose(
                            pt[:, 0:64],
                            vt[h * 64:(h + 1) * 64, s4 * 128:(s4 + 1) * 128],
                            identb[h * 64:(h + 1) * 64, h * 64:(h + 1) * 64])
                        nc.vector.tensor_copy(Vp[:, j, h, 0:64], pt[:, 0:64])
                for s4 in range(4):
                    thunks.append(lambda s4=s4, f=v_tr: f(s4))
            return thunks  # 15 thunks

        def attn_qc(b, qc, fillers=()):
            """One 512-query attention chunk, j-major AV, inline phase C."""
            qcols = slice(b * T + qc * 512, b * T + (qc + 1) * 512)
            po = [pot.tile([65, 512], f32, tag="po", name=f"po{h}")
                  for h in range(HPC)]
            prev_pe = None
            prev_j = -1
            nf = len(fillers)
            fi = 0

            def av(j, pe):
                for h in range(HPC):
                    nc.tensor.matmul(po[h][:], Vp[:, b * NJ + j, h, 0:65],
                                     pe[:, h * 512:(h + 1) * 512],
                                     start=(j == 0), stop=(j == NJ - 1))

            for j in range(NJ):
                scols = slice(b * T + j * 128, b * T + (j + 1) * 128)
                s2 = s2p.tile([128, 1024], f32, tag="s2", name="s2")
                for h in range(HPC):
                    nc.tensor.matmul(s2[:, h * 512:(h + 1) * 512],
                                     KT[h * 64:(h + 1) * 64, scols],
                                     QT[h * 64:(h + 1) * 64, qcols],
                                     start=True, stop=True,
                                     tile_position=(h * 64, 0))
                pe = pp.tile([128, 1024], f32r, tag="p", name="pe")
                if with_mask:
                    mt = mp.tile([128, 512], f32, tag="m", name="mt")
                    nc.sync.dma_start(
                        out=mt[:],
                        in_=maskT.ap()[j * 128:(j + 1) * 128,
                                       qc * 512:(qc + 1) * 512])
                    tmp = tmpp.tile([128, 1024], f32, tag="tmp", name="tmp")
                    for h in range(HPC):
                        nc.vector.scalar_tensor_tensor(
                            tmp[:, h * 512:(h + 1) * 512],
                            s2[:, h * 512:(h + 1) * 512], SCALE, mt[:],
                            op0=ALU.mult, op1=ALU.add)
                    nc.scalar.activation(pe[:], tmp[:], AF.Exp)
                else:
                    nc.scalar.activation(pe[:], s2[:], AF.Exp, scale=float(SCALE))
                if prev_pe is not None:
                    av(prev_j, prev_pe)
                prev_pe, prev_j = pe, j
                want = (j + 1) * nf // NJ
                while fi < want:
                    fillers[fi]()
                    fi += 1
            av(prev_j, prev_pe)

            # phase C inline: po's only reader is the o65 copy, so po frees
            # early; the rest overlaps the next chunk's j-loop via the queues
            for h in range(HPC):
                o65 = o65p.tile([65, 512], f32, tag="o65", name="o65")
                nc.vector.tensor_copy(o65[:], po[h][:])
                drow = rowp.tile([1, 512], f32, tag="row", name="drow")
                nc.sync.dma_start(out=drow[:], in_=o65[64:65, :])
                rrow = rowp.tile([1, 512], f32, tag="row", name="rrow")
                nc.vector.reciprocal(rrow[:], drow[:])
                bc = auxp.tile([HD, 512], f32, tag="bc", name="bc")
                nc.gpsimd.partition_broadcast(bc[:], rrow[:])
                t1 = auxp.tile([HD, 512], f32r, tag="t1", name="t1")
                nc.vector.tensor_tensor(t1[:], o65[0:64, :], bc[:], op=ALU.mult)
                oc = osm2[h * 64:(h + 1) * 64, qcols]
                nc.vector.tensor_scalar_add(oc, t1[:], bv_sb[:, h:h + 1])
            sq2 = auxp.tile([128, 512], f32r, tag="sq", name="sq2")
            nc.vector.tensor_tensor(sq2[:], osm2[:, qcols], osm2[:, qcols],
                                    op=ALU.mult)
            pn = miscp.tile([128, 512], f32, tag="a", name="ps_n")
            nc.tensor.matmul(pn[0:2, :], sel64[:], sq2[:], start=True, stop=True)
            nc.vector.tensor_copy(nsq2[:, qcols], pn[0:2, :])

        def tail():
            s1 = scp.tile([2, NT], f16, tag="s1", bufs=1, name="s1")
            tot2 = scp.tile([2, 1], f32, tag="c1", name="tot2")
            nc.scalar.activation(s1[:], nsq2[:], AF.Sqrt, accum_out=tot2[:])
            den2 = scp.tile([2, 1], f32, tag="c1", name="den2")
            nc.vector.tensor_scalar(den2[:], tot2[:], 1.0 / NT, 1e-5,
                                    op0=ALU.mult, op1=ALU.max)
            inv2 = scp.tile([2, 1], f32, tag="c1", name="inv2")
            nc.vector.reciprocal(inv2[:], den2[:])
            inv2w = scp.tile([128, 512], f16, tag="w2", bufs=1, name="inv2w")
            nc.vector.memset(inv2w[:], 0.0)
            nc.vector.tensor_scalar(inv2w[0:2, :],
                                    ones_f[0:2, :].broadcast_to([2, 512]),
                                    inv2[:], None, op0=ALU.mult)
            pinv = miscp.tile([128, 512], f32, tag="a", name="ps_i")
            nc.tensor.matmul(pinv[:], selbc[:], inv2w[:], start=True, stop=True)
            inv128 = scp.tile([128, 1], f32, tag="c128", name="inv128")
            nc.vector.tensor_copy(inv128[:], pinv[:, 0:1])
            nc.vector.tensor_scalar(wo_sc[:], wo_sb[:], inv128[:],
                                    None, op0=ALU.mult)
            for t in range(NT // 128):
                trows = slice(t * 128, (t + 1) * 128)
                osb = op.tile([128, D], f16, tag="ob", name="osb")
                ppj = s2p.tile([128, 1024], f32, tag="s2", name="ps_p")
                for dchunk in range(2):
                    dcols = slice(dchunk * 512, (dchunk + 1) * 512)
                    nc.tensor.matmul(ppj[:, dcols], osm2[:, trows],
                                     wo_sc[:, dcols], start=True, stop=True)
                if t % 2 == 0:
                    nc.vector.tensor_copy(osb[:], ppj[:])
                else:
                    nc.scalar.activation(osb[:], ppj[:], AF.Copy)
                nc.sync.dma_start(out=outp.ap()[trows, :], in_=osb[:])

        def _emit_all():
            ch = [qkv_thunks(c) for c in range(4)]
            for t in ch[0]:
                t()
            for t in ch[1]:
                t()
            attn_qc(0, 0, fillers=ch[2][:8])
            attn_qc(0, 1, fillers=ch[2][8:])
            attn_qc(0, 2, fillers=ch[3][:8])
            attn_qc(0, 3, fillers=ch[3][8:])
            for qc in range(4):
                attn_qc(1, qc)
            tail()

        if repeat > 1:
            with tc.For_i(0, repeat, 1):
                _emit_all()
        else:
            _emit_all()

    nc.compile()
    return nc


def _get_nc(with_mask: bool):
    key = with_mask
    if key not in _BUILD_CACHE:
        _BUILD_CACHE[key] = _build(with_mask)
    return _BUILD_CACHE[key]


def _make_in_maps(hidden_states, attn_mask, W_q, b_q, W_k, b_k, W_v, b_v,
                  W_o, b_o, gate, with_mask):
    x = hidden_states.reshape(NT, D)
    xT = np.ascontiguousarray(x.T.astype(np.float16))
    g = np.clip(gate, 0.0, 1.0)

    in_maps = []
    for c in range(NCORES):
        hs = slice(c * HPC, (c + 1) * HPC)
        wq = np.concatenate([W_q[c * HPC + i] for i in range(HPC)], axis=1)
        wk = np.concatenate([W_k[c * HPC + i] for i in range(HPC)], axis=1)
        wv = np.concatenate([W_v[c * HPC + i] for i in range(HPC)], axis=1)
        wqkv_c = np.ascontiguousarray(
            np.stack([wq, wk, wv], axis=0).reshape(3, 8, 128, 128)
            .astype(np.float16))
        bqk_c = np.ascontiguousarray(np.stack(
            [np.concatenate([b_q[c * HPC + i] for i in range(HPC)]),
             np.concatenate([b_k[c * HPC + i] for i in range(HPC)])], axis=0))
        bv_c = np.ascontiguousarray(b_v[hs])
        wo_c = np.ascontiguousarray(
            (W_o[hs] * (g[hs, None, None] / H)).astype(np.float16))
        selc_c = np.zeros((128, 128), np.float16)
        selc_c[0, 0:64] = 1.0
        selc_c[1, 64:128] = 1.0
        sel64_c = np.zeros((128, 2), np.float32)
        sel64_c[0:64, 0] = 1.0
        sel64_c[64:128, 1] = 1.0
        m = dict(xT=xT, wqkv=wqkv_c, bqk=bqk_c, bv=bv_c, wo=wo_c,
                 selc=selc_c, sel64d=sel64_c)
        if with_mask:
            m["maskT"] = np.ascontiguousarray(attn_mask.T)
        in_maps.append(m)
    return in_maps


def kernel(hidden_states, attn_mask, W_q, b_q, W_k, b_k, W_v, b_v, W_o, b_o, gate):
    hidden_states = np.asarray(hidden_states, dtype=np.float32)
    attn_mask = np.asarray(attn_mask, dtype=np.float32)
    W_q, b_q = np.asarray(W_q, np.float32), np.asarray(b_q, np.float32)
    W_k, b_k = np.asarray(W_k, np.float32), np.asarray(b_k, np.float32)
    W_v, b_v = np.asarray(W_v, np.float32), np.asarray(b_v, np.float32)
    W_o, b_o = np.asarray(W_o, np.float32), np.asarray(b_o, np.float32)
    gate = np.asarray(gate, np.float32)

    with_mask = bool(np.any(attn_mask))
    nc = _get_nc(with_mask)
    in_maps = _make_in_maps(hidden_states, attn_mask, W_q, b_q, W_k, b_k,
                            W_v, b_v, W_o, b_o, gate, with_mask)

    res = run_bass_kernel_spmd(nc, in_maps, core_ids=list(range(NCORES)))
    if res.exec_time_ns is not None:
        print(f"HW exec time: {res.exec_time_ns} ns")

    out = np.zeros((NT, D), dtype=np.float32)
    for r in res.results:
        out += r["outp"].astype(np.float32)
    b_eff = (np.clip(gate, 0.0, 1.0)[:, None] * b_o).sum(axis=0) / H
    out += b_eff[None, :]
    return out.reshape(B, T, D)


# revision 25
# speedup vs baseline: 2.3574x; 1.1378x over previous
"""Gated multi-head self-attention on 8 Trainium2 NeuronCores.

Sharding: 16 heads / 8 cores = 2 heads per core. Each core computes its two
heads end-to-end and writes a partial [NT, D] output (fp16); the host sums
the 8 partials and adds the head-summed output bias.

Device algorithm per core (heads h0, h1), fp16 matmul front-end, f32 PSUM:
  QT/KT[128, 4096]   = W_{q,k}.T @ x.T + b     (f16; heads stacked on partitions)
  V'[s, h, 66]       = [x@W_v | 1]             (f32r, via VT + PE transpose)
  S^T[s, q]          = KT.T @ QT               (f16 in, heads packed in PE quads)
  P[s, q]            = exp(0.125 * S^T)        (f32r out: Act f16 writes run at
                                                half rate, f32r at full rate)
  po[65, q]          = V'.T @ P                (row 64 = softmax denominators)
  osm[e, q]          = po[0:64]·bcast(1/po[64]) + bv    (f16)
  nsq[q]             = ones.T @ osm^2          (per-token squared norms)
  denom_h            = max(mean sqrt(nsq), 1e-5);  wo_sc = wo * gate/16 / denom
  out               += osm.T @ wo_sc  summed over 2 heads, stored fp16

Scheduling: batch-1 QKV projection is interleaved into batch-0 attention
j-loops as filler thunks so PE stays dense while Act streams the exp()
softmax; phase C overlaps the next chunk's j-loop via the engine queues.
DMA queues: x loads, drow row-moves and output stores all on SP, which is
otherwise idle in those phases.
"""

import sys

sys.path.insert(0, "/opt/trn_rl_repo")

import contextlib

import numpy as np

import concourse.bacc as bacc
import concourse.mybir as mybir
import concourse.tile as tile
from concourse.bass_utils import run_bass_kernel_spmd
from concourse.masks import make_identity

f32 = mybir.dt.float32
f32r = mybir.dt.float32r
f16 = mybir.dt.float16
AF = mybir.ActivationFunctionType
ALU = mybir.AluOpType

B, T, D, H, HD = 2, 2048, 1024, 16, 64
NCORES = 8
HPC = H // NCORES  # heads per core = 2
NT = B * T         # 4096 tokens
NJ = T // 128      # 16 key tiles per batch
SCALE = 1.0 / np.sqrt(HD)  # 0.125

_BUILD_CACHE = {}


def _build(with_mask: bool, repeat: int = 1):
    nc = bacc.Bacc(None, target_bir_lowering=False)

    xT = nc.declare_dram_parameter("xT", [D, NT], f16, isOutput=False)
    wqkv = nc.declare_dram_parameter("wqkv", [3, 8, 128, 128], f16, isOutput=False)
    bqk = nc.declare_dram_parameter("bqk", [2, 128], f32, isOutput=False)
    bv = nc.declare_dram_parameter("bv", [HPC, HD], f32, isOutput=False)
    wo = nc.declare_dram_parameter("wo", [HPC, HD, D], f16, isOutput=False)
    selc = nc.declare_dram_parameter("selc", [128, 128], f16, isOutput=False)
    sel64d = nc.declare_dram_parameter("sel64d", [128, 2], f32r, isOutput=False)
    outp = nc.declare_dram_parameter("outp", [NT, D], f16, isOutput=True)
    if with_mask:
        maskT = nc.declare_dram_parameter("maskT", [T, T], f32, isOutput=False)

    with tile.TileContext(nc) as tc, contextlib.ExitStack() as ctx:
        wp = ctx.enter_context(tc.tile_pool(name="wp", bufs=1))
        big = ctx.enter_context(tc.tile_pool(name="big", bufs=2))
        osmp = ctx.enter_context(tc.tile_pool(name="osmp", bufs=2))
        nsqp = ctx.enter_context(tc.tile_pool(name="nsqp", bufs=2))
        xp = ctx.enter_context(tc.tile_pool(name="xp", bufs=4))
        vtp = ctx.enter_context(tc.tile_pool(name="vtp", bufs=2))
        pp = ctx.enter_context(tc.tile_pool(name="pp", bufs=4))
        o65p = ctx.enter_context(tc.tile_pool(name="o65p", bufs=4))
        auxp = ctx.enter_context(tc.tile_pool(name="auxp", bufs=4))
        rowp = ctx.enter_context(tc.tile_pool(name="rowp", bufs=4))
        scp = ctx.enter_context(tc.tile_pool(name="scp", bufs=4))
        op = ctx.enter_context(tc.tile_pool(name="op", bufs=5))
        if with_mask:
            mp = ctx.enter_context(tc.tile_pool(name="mp", bufs=2))
            tmpp = ctx.enter_context(tc.tile_pool(name="tmpp", bufs=2))
        s2p = ctx.enter_context(tc.tile_pool(name="s2p", bufs=2, space="PSUM"))
        pot = ctx.enter_context(tc.tile_pool(name="pot", bufs=2, space="PSUM"))
        miscp = ctx.enter_context(tc.tile_pool(name="miscp", bufs=2, space="PSUM"))

        # ---- constants / weights ----
        wqkv_sb = wp.tile([128, 3, 8, 128], f16)
        nc.sync.dma_start(out=wqkv_sb[:], in_=wqkv.ap().rearrange("q d p m -> p q d m"))
        bqk_sb = wp.tile([128, 2], f32)
        nc.sync.dma_start(out=bqk_sb[:], in_=bqk.ap().rearrange("q p -> p q"))
        bv_sb = wp.tile([HD, HPC], f32)
        nc.sync.dma_start(out=bv_sb[:], in_=bv.ap().rearrange("h p -> p h"))
        wo_sb = wp.tile([128, D], f16)
        nc.sync.dma_start(out=wo_sb[:], in_=wo.ap().rearrange("h p d -> (h p) d"))
        wo_sc = wp.tile([128, D], f16)
        ones_f = wp.tile([128, 1], f32)
        nc.vector.memset(ones_f[:], 1.0)
        sel64 = wp.tile([128, 2], f32r)
        nc.sync.dma_start(out=sel64[:], in_=sel64d.ap())
        selbc = wp.tile([128, 128], f16)
        nc.sync.dma_start(out=selbc[:], in_=selc.ap())
        identb = wp.tile([128, 128], f16)
        make_identity(nc, identb[:])

        # V' [s-part, s-tile, head, 66]: cols 0:64 = V, col 64 = ones, 65 pad
        Vp = wp.tile([128, NT // 128, HPC, 66], f32r)
        nc.vector.tensor_copy(Vp[:, :, :, 64:65],
                              ones_f.broadcast_to([128, NT // 128, HPC, 1]))

        QT = big.tile([128, NT], f16, tag="big")
        KT = big.tile([128, NT], f16, tag="big")

        osm2 = osmp.tile([128, NT], f16, name="osm2", tag="osm")
        nsq2 = nsqp.tile([2, NT], f16, name="nsq2", tag="nsq")

        def qkv_thunks(c4):
            """Emit-thunks projecting tokens [c4*1024, (c4+1)*1024)."""
            thunks = []
            xsh = {}

            def dma_thunk():
                for hh in range(2):
                    xs = xp.tile([128, 4, 1024], f16, tag="xs", name=f"xs{hh}")
                    for dd in range(4):
                        nc.sync.dma_start(
                            out=xs[:, dd, :],
                            in_=xT.ap()[:, c4 * 1024:(c4 + 1) * 1024]
                            .rearrange("(dc p) t -> p dc t", p=128)[:, hh * 4 + dd, :])
                    xsh[hh] = xs
            thunks.append(dma_thunk)

            def xsl(dc, cols):
                return xsh[dc // 4][:, dc % 4, cols]

            for sub in range(2):
                scols = slice(sub * 512, (sub + 1) * 512)
                gcols = slice(c4 * 1024 + sub * 512, c4 * 1024 + (sub + 1) * 512)

                def qk_proj(p, scols=scols, gcols=gcols):
                    dst = QT if p == 0 else KT
                    ps = miscp.tile([128, 512], f32, tag="a", name="ps_qk")
                    for dc in range(8):
                        nc.tensor.matmul(ps[:], wqkv_sb[:, p, dc, :], xsl(dc, scols),
                                         start=(dc == 0), stop=(dc == 7))
                    nc.vector.tensor_scalar_add(dst[:, gcols], ps[:],
                                                bqk_sb[:, p:p + 1])
                thunks.append(lambda p=0, f=qk_proj: f(p))
                thunks.append(lambda p=1, f=qk_proj: f(p))

                def v_proj(c4=c4, sub=sub, scols=scols):
                    psv = miscp.tile([128, 512], f32, tag="a", name="ps_v")
                    for dc in range(8):
                        nc.tensor.matmul(psv[:], wqkv_sb[:, 2, dc, :], xsl(dc, scols),
                                         start=(dc == 0), stop=(dc == 7))
                    vt = vtp.tile([128, 512], f16, tag="vt", name="vt")
                    nc.vector.tensor_copy(vt[:], psv[:])
                    xsh[f"vt{sub}"] = vt
                thunks.append(v_proj)

                def v_tr(s4, c4=c4, sub=sub):
                    vt = xsh[f"vt{sub}"]
                    j = c4 * 8 + sub * 4 + s4
                    for h in range(HPC):
                        pt = miscp.tile([128, 1024], f16, tag="a", name="ps_tr")
                        nc.tensor.transpose(
                            pt[:, 0:64],
                            vt[h * 64:(h + 1) * 64, s4 * 128:(s4 + 1) * 128],
                            identb[h * 64:(h + 1) * 64, h * 64:(h + 1) * 64])
                        nc.vector.tensor_copy(Vp[:, j, h, 0:64], pt[:, 0:64])
                for s4 in range(4):
                    thunks.append(lambda s4=s4, f=v_tr: f(s4))
            return thunks  # 15 thunks

        def attn_qc(b, qc, fillers=()):
            """One 512-query attention chunk, j-major AV, inline phase C."""
            qcols = slice(b * T + qc * 512, b * T + (qc + 1) * 512)
            po = [pot.tile([65, 512], f32, tag="po", name=f"po{h}")
                  for h in range(HPC)]
            prev_pe = None
            prev_j = -1
            nf = len(fillers)
            fi = 0

            def av(j, pe):
                for h in range(HPC):
                    nc.tensor.matmul(po[h][:], Vp[:, b * NJ + j, h, 0:65],
                                     pe[:, h * 512:(h + 1) * 512],
                                     start=(j == 0), stop=(j == NJ - 1))

            for j in range(NJ):
                scols = slice(b * T + j * 128, b * T + (j + 1) * 128)
                s2 = s2p.tile([128, 1024], f32, tag="s2", name="s2")
                for h in range(HPC):
                    nc.tensor.matmul(s2[:, h * 512:(h + 1) * 512],
                                     KT[h * 64:(h + 1) * 64, scols],
                                     QT[h * 64:(h + 1) * 64, qcols],
                                     start=True, stop=True,
                                     tile_position=(h * 64, 0))
                pe = pp.tile([128, 1024], f32r, tag="p", name="pe")
                if with_mask:
                    mt = mp.tile([128, 512], f32, tag="m", name="mt")
                    nc.sync.dma_start(
                        out=mt[:],
                        in_=maskT.ap()[j * 128:(j + 1) * 128,
                                       qc * 512:(qc + 1) * 512])
                    tmp = tmpp.tile([128, 1024], f32, tag="tmp", name="tmp")
                    for h in range(HPC):
                        nc.vector.scalar_tensor_tensor(
                            tmp[:, h * 512:(h + 1) * 512],
                            s2[:, h * 512:(h + 1) * 512], SCALE, mt[:],
                            op0=ALU.mult, op1=ALU.add)
                    nc.scalar.activation(pe[:], tmp[:], AF.Exp)
                else:
                    nc.scalar.activation(pe[:], s2[:], AF.Exp, scale=float(SCALE))
                if prev_pe is not None:
                    av(prev_j, prev_pe)
                prev_pe, prev_j = pe, j
                want = (j + 1) * nf // NJ
                while fi < want:
                    fillers[fi]()
                    fi += 1
            av(prev_j, prev_pe)

            # phase C inline: po's only reader is the o65 copy, so po frees
            # early; the rest overlaps the next chunk's j-loop via the queues
            for h in range(HPC):
                o65 = o65p.tile([65, 512], f32, tag="o65", name="o65")
                nc.vector.tensor_copy(o65[:], po[h][:])
                drow = rowp.tile([1, 512], f32, tag="row", name="drow")
                nc.sync.dma_start(out=drow[:], in_=o65[64:65, :])
                rrow = rowp.tile([1, 512], f32, tag="row", name="rrow")
                nc.vector.reciprocal(rrow[:], drow[:])
                bc = auxp.tile([HD, 512], f32, tag="bc", name="bc")
                nc.gpsimd.partition_broadcast(bc[:], rrow[:])
                t1 = auxp.tile([HD, 512], f32r, tag="t1", name="t1")
                nc.vector.tensor_tensor(t1[:], o65[0:64, :], bc[:], op=ALU.mult)
                oc = osm2[h * 64:(h + 1) * 64, qcols]
                nc.vector.tensor_scalar_add(oc, t1[:], bv_sb[:, h:h + 1])
            sq2 = auxp.tile([128, 512], f32r, tag="sq", name="sq2")
            nc.vector.tensor_tensor(sq2[:], osm2[:, qcols], osm2[:, qcols],
                                    op=ALU.mult)
            pn = miscp.tile([128, 512], f32, tag="a", name="ps_n")
            nc.tensor.matmul(pn[0:2, :], sel64[:], sq2[:], start=True, stop=True)
            nc.vector.tensor_copy(nsq2[:, qcols], pn[0:2, :])

        def tail():
            s1 = scp.tile([2, NT], f16, tag="s1", bufs=1, name="s1")
            tot2 = scp.tile([2, 1], f32, tag="c1", name="tot2")
            nc.scalar.activation(s1[:], nsq2[:], AF.Sqrt, accum_out=tot2[:])
            den2 = scp.tile([2, 1], f32, tag="c1", name="den2")
            nc.vector.tensor_scalar(den2[:], tot2[:], 1.0 / NT, 1e-5,
                                    op0=ALU.mult, op1=ALU.max)
            inv2 = scp.tile([2, 1], f32, tag="c1", name="inv2")
            nc.vector.reciprocal(inv2[:], den2[:])
            inv2w = scp.tile([128, 512], f16, tag="w2", bufs=1, name="inv2w")
            nc.vector.memset(inv2w[:], 0.0)
            nc.vector.tensor_scalar(inv2w[0:2, :],
                                    ones_f[0:2, :].broadcast_to([2, 512]),
                                    inv2[:], None, op0=ALU.mult)
            pinv = miscp.tile([128, 512], f32, tag="a", name="ps_i")
            nc.tensor.matmul(pinv[:], selbc[:], inv2w[:], start=True, stop=True)
            inv128 = scp.tile([128, 1], f32, tag="c128", name="inv128")
            nc.vector.tensor_copy(inv128[:], pinv[:, 0:1])
            nc.vector.tensor_scalar(wo_sc[:], wo_sb[:], inv128[:],
                                    None, op0=ALU.mult)
            for t in range(NT // 128):
                trows = slice(t * 128, (t + 1) * 128)
                osb = op.tile([128, D], f16, tag="ob", name="osb")
                if t % 2 == 0:
                    big_ppj = s2p.tile([128, 1024], f32, tag="s2", name="ps_p")
                    halves = [big_ppj[:, 0:512], big_ppj[:, 512:1024]]
                else:
                    halves = [miscp.tile([128, 512], f32, tag="a",
                                         name=f"ps_p{d}")[:]
                              for d in range(2)]
                for dchunk in range(2):
                    dcols = slice(dchunk * 512, (dchunk + 1) * 512)
                    ppj = halves[dchunk]
                    nc.tensor.matmul(ppj, osm2[:, trows],
                                     wo_sc[:, dcols], start=True, stop=True)
                    # per-half copy+store: twice the chunks in flight; DVE
                    # gets the larger share (f16 copies: DVE 1.4us vs Act 2us)
                    if (2 * t + dchunk) % 5 < 3:
                        nc.vector.tensor_copy(osb[:, dcols], ppj)
                    else:
                        nc.scalar.activation(osb[:, dcols], ppj, AF.Copy)
                nc.sync.dma_start(out=outp.ap()[trows, :], in_=osb[:])

        def _emit_all():
            ch = [qkv_thunks(c) for c in range(4)]
            for t in ch[0]:
                t()
            ch[1][0]()  # prefetch chunk-1 x before attention starts
            attn_qc(0, 0, fillers=ch[1][1:] + [ch[2][0]])
            attn_qc(0, 1, fillers=ch[2][1:] + [ch[3][0]])
            attn_qc(0, 2, fillers=ch[3][1:])
            attn_qc(0, 3)
            for qc in range(4):
                attn_qc(1, qc)
            tail()

        if repeat > 1:
            with tc.For_i(0, repeat, 1):
                _emit_all()
        else:
            _emit_all()

    nc.compile()
    return nc


def _get_nc(with_mask: bool):
    key = with_mask
    if key not in _BUILD_CACHE:
        _BUILD_CACHE[key] = _build(with_mask)
    return _BUILD_CACHE[key]


def _make_in_maps(hidden_states, attn_mask, W_q, b_q, W_k, b_k, W_v, b_v,
                  W_o, b_o, gate, with_mask):
    x = hidden_states.reshape(NT, D)
    xT = np.ascontiguousarray(x.T.astype(np.float16))
    g = np.clip(gate, 0.0, 1.0)

    in_maps = []
    for c in range(NCORES):
        hs = slice(c * HPC, (c + 1) * HPC)
        wq = np.concatenate([W_q[c * HPC + i] for i in range(HPC)], axis=1)
        wk = np.concatenate([W_k[c * HPC + i] for i in range(HPC)], axis=1)
        wv = np.concatenate([W_v[c * HPC + i] for i in range(HPC)], axis=1)
        wqkv_c = np.ascontiguousarray(
            np.stack([wq, wk, wv], axis=0).reshape(3, 8, 128, 128)
            .astype(np.float16))
        bqk_c = np.ascontiguousarray(np.stack(
            [np.concatenate([b_q[c * HPC + i] for i in range(HPC)]),
             np.concatenate([b_k[c * HPC + i] for i in range(HPC)])], axis=0))
        bv_c = np.ascontiguousarray(b_v[hs])
        wo_c = np.ascontiguousarray(
            (W_o[hs] * (g[hs, None, None] / H)).astype(np.float16))
        selc_c = np.zeros((128, 128), np.float16)
        selc_c[0, 0:64] = 1.0
        selc_c[1, 64:128] = 1.0
        sel64_c = np.zeros((128, 2), np.float32)
        sel64_c[0:64, 0] = 1.0
        sel64_c[64:128, 1] = 1.0
        m = dict(xT=xT, wqkv=wqkv_c, bqk=bqk_c, bv=bv_c, wo=wo_c,
                 selc=selc_c, sel64d=sel64_c)
        if with_mask:
            m["maskT"] = np.ascontiguousarray(attn_mask.T)
        in_maps.append(m)
    return in_maps


def kernel(hidden_states, attn_mask, W_q, b_q, W_k, b_k, W_v, b_v, W_o, b_o, gate):
    hidden_states = np.asarray(hidden_states, dtype=np.float32)
    attn_mask = np.asarray(attn_mask, dtype=np.float32)
    W_q, b_q = np.asarray(W_q, np.float32), np.asarray(b_q, np.float32)
    W_k, b_k = np.asarray(W_k, np.float32), np.asarray(b_k, np.float32)
    W_v, b_v = np.asarray(W_v, np.float32), np.asarray(b_v, np.float32)
    W_o, b_o = np.asarray(W_o, np.float32), np.asarray(b_o, np.float32)
    gate = np.asarray(gate, np.float32)

    with_mask = bool(np.any(attn_mask))
    nc = _get_nc(with_mask)
    in_maps = _make_in_maps(hidden_states, attn_mask, W_q, b_q, W_k, b_k,
                            W_v, b_v, W_o, b_o, gate, with_mask)

    res = run_bass_kernel_spmd(nc, in_maps, core_ids=list(range(NCORES)))
    if res.exec_time_ns is not None:
        print(f"HW exec time: {res.exec_time_ns} ns")

    out = np.zeros((NT, D), dtype=np.float32)
    for r in res.results:
        out += r["outp"].astype(np.float32)
    b_eff = (np.clip(gate, 0.0, 1.0)[:, None] * b_o).sum(axis=0) / H
    out += b_eff[None, :]
    return out.reshape(B, T, D)
